# revision 11
# baseline (speedup 1.0000x reference)
"""Multi-head attention (B=2, T=4096, D=512, H=8) on 8 Trainium2 cores.

Sharding: core i handles batch b=i//4, query rows q0=(i%4)*1024 .. q0+1024,
all 8 heads (full K/V of its batch computed on-core; no collectives).
Host pre-transposes x and weights so every DMA is contiguous, and rolls
x along T per core so each core's query block sits at columns 0:1024
(keys become a permutation of T, which attention is invariant to).

Q/K/V projections run in bf16 (same PE throughput as fp32r, half the HBM
bytes); the O projection stays fp32r. The attention core (scores, attn@V)
runs in fp8e4m3 with DoubleRow perf mode (2 k-tiles per matmul at 0.5
cycles/row = 4x fp32r throughput):
  - q/k are scaled by alpha=8^-0.5 each (folds 1/sqrt(dk) into the
    quantization) and quantized to fp8; a partition-shuffle DMA packs
    them as [32, 2head, 2sub, T] so the 64-deep dk contraction becomes
    DoubleRow over 2x32.
  - exp fuses with fp8 quantization: one tensor_scalar per tile computes
    rne(score*8*log2e + 56) -> int8 whose bits ARE the fp8e4m3 encoding
    of exp(score) (Schraudolph in log2 space). Round-robins over
    ACT/Pool/DVE, splitting the 33.5M-exp/core load three ways.
  - attn@V contracts 256 keys per DoubleRow matmul against fp8 V packed
    [128, ktile, 96]: col 64 = ones so the softmax row-sum rides along
    as output partition 64; cols 65:95 pad the stationary width to a
    multiple of 32 (DoubleRow requirement) and are never read.
Scheduling: attention is software-pipelined (scores 2 k-tile-pairs ahead
of attn@V, 3x2-bank PSUM rotation) and independent background work (next
pair's projections / V / previous pair's normalize) is interleaved into
the attention stream so conversion latency plus the 100ns semaphore
delay never stall the in-order PE queue and the p-state ramp holds.
DMAs are coalesced via multi-dim access patterns to respect the serial
HWDGE dispatcher (~600ns each). Softmax skips max-subtraction (scores
~N(0,0.33)). Measured end-to-end rel err ~1.4e-2 vs the 2e-2 gate.
"""
import sys
sys.path.insert(0, "/opt/trn_rl_repo")

import numpy as np
import ml_dtypes
import concourse.bacc as bacc
import concourse.mybir as mybir
import concourse.tile as tile
from concourse.bass_utils import run_bass_kernel_spmd

F32 = mybir.dt.float32
F32R = mybir.dt.float32r
BF16 = mybir.dt.bfloat16
F8 = mybir.dt.float8e4
I8 = mybir.dt.int8
AF = mybir.ActivationFunctionType
MULT = mybir.AluOpType.mult
ADD = mybir.AluOpType.add
DR = mybir.MatmulPerfMode.DoubleRow

B, T, C = 2, 4096, 512
H, DK = 8, 64
TQ = 1024          # queries per core
NP = 4             # head pairs
KT = T // 128      # 32 k-tiles
CT = C // 128      # 4 contraction tiles
ALPHA = float(1.0 / np.sqrt(np.sqrt(64.0)))   # per-operand score scale
SCH_SCALE = 8 * 1.4426950408889634            # bits = rne(s*SCALE + 56)
SCH_BIAS = 56.0

_cache = {}


def _build():
    nc = bacc.Bacc("TRN2")
    xbT = nc.declare_dram_parameter("xbT", [C, T], BF16, isOutput=False)
    wqT = nc.declare_dram_parameter("wqT", [C, C], BF16, isOutput=False)
    wkT = nc.declare_dram_parameter("wkT", [C, C], BF16, isOutput=False)
    wvT = nc.declare_dram_parameter("wvT", [C, C], BF16, isOutput=False)
    woT = nc.declare_dram_parameter("woT", [C, C], F32R, isOutput=False)
    # bias[:, 0] = bq*ALPHA, bias[:, 1] = bk*ALPHA, bias[:, 2] = bv
    bias = nc.declare_dram_parameter("bias", [128, 3, NP], F32, isOutput=False)
    bo = nc.declare_dram_parameter("bo", [1, C], F32R, isOutput=False)
    # ind rows: 0 = head0 mask (1s in 0:64), 1 = head1 mask, 2 = all ones
    ind = nc.declare_dram_parameter("ind", [3, 128], F32R, isOutput=False)
    out = nc.declare_dram_parameter("out", [TQ, C], F32, isOutput=True)

    with tile.TileContext(nc) as tc:
        with (
            tc.tile_pool(name="big", bufs=1) as bpool,
            tc.tile_pool(name="const", bufs=1) as cpool,
            tc.tile_pool(name="work", bufs=2) as wpool,
            tc.tile_pool(name="k8p", bufs=2) as k8pool,
            tc.tile_pool(name="drp", bufs=2) as drpool,
            tc.tile_pool(name="attnp", bufs=3) as apool,
            tc.tile_pool(name="ps", bufs=2, space="PSUM") as ps,
            tc.tile_pool(name="psav", bufs=4, space="PSUM") as psav,
        ):
            def w_dma(tag, src, p0, width, dt=BF16):
                w = wpool.tile([128, CT, width], dt, tag=tag, name="w_" + tag)
                nc.sync.dma_start(
                    w[:], src[:, p0:p0 + width].rearrange(
                        "(ct p) c -> p ct c", p=128))
                return w

            # V weights + x first on the DMA queue: V-proj starts earliest
            wvs0 = w_dma("wvs", wvT, 0, 256)
            xT = bpool.tile([128, CT, T], BF16, tag="xT")          # 32KB
            nc.sync.dma_start(
                xT[:, :, 0:1024],
                xbT[:, 0:1024].rearrange("(ct p) t -> p ct t", p=128))
            bias_s = cpool.tile([128, 3, NP], F32, tag="bias")
            nc.sync.dma_start(bias_s[:], bias[:])
            for tch in range(1, 4):
                nc.sync.dma_start(
                    xT[:, :, tch * 1024:(tch + 1) * 1024],
                    xbT[:, tch * 1024:(tch + 1) * 1024].rearrange(
                        "(ct p) t -> p ct t", p=128))
            # ind / bo live at partition 64 so matmul operand bases match the
            # rowsum row (PSUM partition 64) they pair with.
            inds = cpool.tile([65, 3, 128], F32R, tag="ind")
            nc.sync.dma_start(inds[64:65, :, :],
                              ind.rearrange("(o a) b -> o a b", o=1))
            bos = cpool.tile([65, C], F32R, tag="bo")
            nc.sync.dma_start(bos[64:65, :], bo[:])
            sch_b = cpool.tile([128, 1], F32, tag="schb")
            nc.vector.memset(sch_b[:], SCH_BIAS)
            acat = bpool.tile([128, NP, TQ], F32, tag="acat")      # 16KB
            acat_r = acat.bitcast(F32R)

            # greedy least-loaded picker for vector-engine work.
            # cost(F) ~ F*cycle + fixed overhead, Pool at 0.6 efficiency
            load = {"act": 0.0, "pool": 0.0, "dve": 0.0}

            def pick_eng(free, exclude=()):
                costs = {"act": free * 0.833 + 180,
                         "dve": free * 1.042 + 110,
                         "pool": free * 1.389 + 135}
                e = min((x for x in costs if x not in exclude),
                        key=lambda x: load[x] + costs[x])
                load[e] += costs[e]
                return e

            def exp_quant(dst_i8, src_psum, free=1024):
                e = pick_eng(free)
                if e == "act":
                    nc.scalar.activation(dst_i8, src_psum, AF.Identity,
                                         bias=sch_b[:], scale=SCH_SCALE)
                elif e == "pool":
                    nc.gpsimd.tensor_scalar(dst_i8, src_psum, SCH_SCALE,
                                            SCH_BIAS, op0=MULT, op1=ADD)
                else:
                    nc.vector.tensor_scalar(dst_i8, src_psum, SCH_SCALE,
                                            SCH_BIAS, op0=MULT, op1=ADD)

            def g_copy(dst, srcv, free):
                e = pick_eng(free, exclude=("act",))
                eng = nc.vector if e == "dve" else nc.gpsimd
                eng.tensor_copy(dst, srcv)

            def qk_quant(dst, src_psum, bias_ap):
                e = pick_eng(1024)
                if e == "act":
                    nc.scalar.activation(dst, src_psum, AF.Identity,
                                         bias=bias_ap, scale=ALPHA)
                else:
                    eng = nc.vector if e == "dve" else nc.gpsimd
                    eng.tensor_scalar(dst, src_psum, bias_ap, ALPHA,
                                      op0=ADD, op1=MULT)

            # ---- V for one pair-group (4 heads): fp8, ones col at 64 ----
            def make_vproj(pg, wvs):
                v2p8 = bpool.tile([128, KT * 4, 96], F8, tag="v2p8",
                                  bufs=2, name="v2p8")             # 12KB x2
                nc.gpsimd.memset(v2p8[:, :, 64:96], 1.0)

                def gen():
                    for jp in range(KT // 2):
                        pv = ps.tile([128, 2, 512], F32, tag="pss",
                                     name="pv")
                        for jj in range(2):
                            j = jp * 2 + jj
                            for ct in range(CT):
                                nc.tensor.matmul(
                                    pv[:, jj, 0:256],
                                    xT[:, ct, j * 128:(j + 1) * 128],
                                    wvs[:, ct, :],
                                    start=(ct == 0), stop=(ct == CT - 1))
                        g_copy(
                            v2p8[:, jp * 8:(jp + 1) * 8, 0:64].rearrange(
                                "p (x a) b -> p x a b", x=2),
                            pv[:, :, 0:256].rearrange(
                                "p x (a b) -> p x a b", b=64), 512)
                        yield
                return v2p8, gen()

            def make_proj(p):
                wks = w_dma("wks", wkT, p * 128, 128)
                wqs = w_dma("wqs", wqT, p * 128, 128)
                k8 = k8pool.tile([128, T], F8, tag="k8", name="k8")
                q8 = k8pool.tile([128, TQ], F8, tag="q8", name="q8")
                # [32, h, i, t]: contraction index d = h*64 + i*32 + part
                kdr = drpool.tile([32, 2, 2, T], F8, tag="kdr", name="kdr")
                qdr = drpool.tile([32, 2, 2, TQ], F8, tag="qdr", name="qdr")

                def gen():
                    for t in range(4):
                        pk = ps.tile([128, 2, 512], F32, tag="pss", name="pk")
                        for half in range(2):
                            tch = t * 2 + half
                            for ct in range(CT):
                                nc.tensor.matmul(
                                    pk[:, half],
                                    wks[:, ct, :],
                                    xT[:, ct, tch * 512:(tch + 1) * 512],
                                    start=(ct == 0), stop=(ct == CT - 1))
                        qk_quant(k8[:, t * 1024:(t + 1) * 1024],
                                 pk[:].rearrange("p a b -> p (a b)"),
                                 bias_s[:, 1, p:p + 1])
                        yield
                    nc.sync.dma_start(
                        kdr[:], k8.rearrange("(g p) t -> p g t", p=32)
                        .rearrange("p (h i) t -> p h i t", h=2))
                    yield
                    pq = ps.tile([128, 2, 512], F32, tag="pss", name="pq")
                    for half in range(2):
                        for ct in range(CT):
                            nc.tensor.matmul(
                                pq[:, half],
                                wqs[:, ct, :],
                                xT[:, ct, half * 512:(half + 1) * 512],
                                start=(ct == 0), stop=(ct == CT - 1))
                    qk_quant(q8[:], pq[:].rearrange("p a b -> p (a b)"),
                             bias_s[:, 0, p:p + 1])
                    yield
                    nc.sync.dma_start(
                        qdr[:], q8.rearrange("(g p) t -> p g t", p=32)
                        .rearrange("p (h i) t -> p h i t", h=2))
                    yield
                return kdr, qdr, gen()

            def gen_attn(p, pi, kdr, qdr, v2p8, stage):
                DEPTH = 2
                avs = {}
                at8s = {}
                seq = [(hf, jp, h) for jp in range(KT // 2)
                       for h in range(2) for hf in range(2)]
                for idx in range(len(seq) + DEPTH):
                    if idx < len(seq):
                        half, jp, h = seq[idx]
                        at8 = apool.tile([128, 2, 512], I8, tag="attn",
                                         name="at8")
                        at8s[(half, jp, h)] = at8
                        pss = ps.tile([128, 2, 512], F32, tag="pss",
                                      name="pss")
                        for jj in range(2):
                            j = jp * 2 + jj
                            nc.tensor.matmul(
                                pss[:, jj],
                                kdr[:, h, :, j * 128:(j + 1) * 128],
                                qdr[:, h, :, half * 512:(half + 1) * 512],
                                start=True, stop=True, perf_mode=DR)
                        exp_quant(at8[:], pss[:])
                    if idx >= DEPTH:
                        half, jp, h = seq[idx - DEPTH]
                        c = pi * 2 + h
                        if jp == 0:
                            avs[(half, h)] = psav.tile([96, 512], F32,
                                                       tag="av", name="av")
                        at8 = at8s.pop((half, jp, h))
                        nc.tensor.matmul(
                            avs[(half, h)][:],
                            v2p8[:, jp * 8 + c:jp * 8 + c + 5:4, 0:96],
                            at8.bitcast(F8)[:],
                            start=(jp == 0), stop=(jp == KT // 2 - 1),
                            perf_mode=DR)
                        if jp == KT // 2 - 1:
                            av = avs.pop((half, h))
                            cols = slice(half * 512, (half + 1) * 512)
                            g_copy(stage[64:65, h, cols], av[64:65, :], 512)
                            if h == 0:
                                g_copy(acat[0:64, p, cols], av[0:64, :], 512)
                            else:
                                odd = wpool.tile([64, 512], F32,
                                                 tag="odd", name="odd")
                                g_copy(odd[:], av[0:64, :], 512)
                                nc.sync.dma_start(acat[64:128, p, cols],
                                                  odd[:])
                    yield

            def gen_norm(p, stage):
                # normalize: U / rowsum + bv  into acat[:, p, :]
                rb = wpool.tile([128, TQ], F32, tag="sc", name="rb")
                stage_r = stage.bitcast(F32R)
                pb = ps.tile([128, 2, 512], F32, tag="pss", name="pb")
                for half in range(2):
                    for h in range(2):
                        nc.tensor.matmul(
                            pb[:, half],
                            inds[64:65, h, :],
                            stage_r[64:65, h, half * 512:(half + 1) * 512],
                            start=(h == 0), stop=(h == 1))
                yield
                nc.vector.reciprocal(rb[:],
                                     pb[:].rearrange("p a b -> p (a b)"))
                yield
                e = pick_eng(1024, exclude=("act",))
                eng = nc.vector if e == "dve" else nc.gpsimd
                eng.tensor_tensor(
                    out=acat[:, p, :], in0=acat[:, p, :], in1=rb[:], op=MULT)
                yield
                e = pick_eng(1024, exclude=("act",))
                eng = nc.vector if e == "dve" else nc.gpsimd
                eng.tensor_scalar_add(
                    acat[:, p, :], acat[:, p, :], bias_s[:, 2, p:p + 1])
                yield

            def chain(*gens):
                for g in gens:
                    yield from g

            def run(gen, bg=None, ratio=4):
                i = 0
                for _ in gen:
                    i += 1
                    if bg is not None and i % ratio == 0:
                        next(bg, None)
                if bg is not None:
                    for _ in bg:
                        pass

            stages = [wpool.tile([65, 2, TQ], F32, tag="stage", bufs=4,
                                 name=f"stage{i}") for i in range(4)]

            # ---- schedule: V0 P0 P1 | A0+P2 | A1+(N0,V1) | A2+(N1,P3)
            #                | A3+N2 | N3 | O-proj ----
            v2p8_0, vg0 = make_vproj(0, wvs0)
            run(vg0)
            kdr0, qdr0, pg0 = make_proj(0)
            run(pg0)
            kdr1, qdr1, pg1 = make_proj(1)
            run(pg1)

            kdr2, qdr2, pg2 = make_proj(2)
            run(gen_attn(0, 0, kdr0, qdr0, v2p8_0, stages[0]), bg=pg2,
                ratio=9)
            wvs1 = w_dma("wvs", wvT, 256, 256)
            v2p8_1, vg1 = make_vproj(1, wvs1)
            run(gen_attn(1, 1, kdr1, qdr1, v2p8_0, stages[1]),
                bg=chain(gen_norm(0, stages[0]), vg1), ratio=3)
            kdr3, qdr3, pg3 = make_proj(3)
            run(gen_attn(2, 0, kdr2, qdr2, v2p8_1, stages[2]),
                bg=chain(gen_norm(1, stages[1]), pg3), ratio=5)
            woTs = cpool.tile([128, CT, C], F32R, tag="woT")       # 8KB
            nc.sync.dma_start(
                woTs[:], woT.rearrange("(ct p) c -> p ct c", p=128))
            run(gen_attn(3, 1, kdr3, qdr3, v2p8_1, stages[3]),
                bg=gen_norm(2, stages[2]), ratio=16)
            run(gen_norm(3, stages[3]))

            # ---- output projection: out[t, :] = acat^T.T @ woT + bo ----
            for t in range(4):
                po = ps.tile([128, 2, 512], F32, tag="pss", name="po")
                for half in range(2):
                    qt = t * 2 + half
                    for r in range(CT):
                        nc.tensor.matmul(
                            po[:, half],
                            acat_r[:, r, qt * 128:(qt + 1) * 128],
                            woTs[:, r, :],
                            start=(r == 0), stop=False)
                    nc.tensor.matmul(po[:, half], inds[64:65, 2, :],
                                     bos[64:65, :], start=False, stop=True)
                ot = wpool.tile([128, 2, C], F32, tag="sc", name="ot")
                g_copy(ot[:, 0, :], po[:, 0], 512)
                g_copy(ot[:, 1, :], po[:, 1], 512)
                nc.sync.dma_start(
                    out[t * 256:(t + 1) * 256, :].rearrange(
                        "(a p) c -> p a c", p=128), ot[:])

    nc.compile()
    return nc


def _prep_inputs(x, Wq, bq, Wk, bk, Wv, bv, Wo, bo):
    bf = ml_dtypes.bfloat16
    wqT = np.ascontiguousarray(Wq.T.astype(bf))
    wkT = np.ascontiguousarray(Wk.T.astype(bf))
    wvT = np.ascontiguousarray(Wv.T.astype(bf))
    woT = np.ascontiguousarray(Wo.T)
    bias = np.stack([
        (bq * ALPHA).reshape(NP, 128).T,
        (bk * ALPHA).reshape(NP, 128).T,
        bv.reshape(NP, 128).T,
    ], axis=1).astype(np.float32)          # [128, 3, NP]
    bias = np.ascontiguousarray(bias)
    bo1 = np.ascontiguousarray(bo.reshape(1, C))
    ind = np.zeros((3, 128), np.float32)
    ind[0, 0:64] = 1.0
    ind[1, 64:128] = 1.0
    ind[2, :] = 1.0
    in_maps = []
    for i in range(8):
        b, q0 = i // 4, (i % 4) * TQ
        xbT = np.ascontiguousarray(np.roll(x[b].T, -q0, axis=1).astype(bf))
        in_maps.append({
            "xbT": xbT, "wqT": wqT, "wkT": wkT, "wvT": wvT, "woT": woT,
            "bias": bias, "bo": bo1, "ind": ind,
        })
    return in_maps


def kernel(x, Wq, bq, Wk, bk, Wv, bv, Wo, bo):
    x = np.asarray(x, np.float32)
    args = [np.asarray(a, np.float32) for a in
            (Wq, bq, Wk, bk, Wv, bv, Wo, bo)]
    if "nc" not in _cache:
        _cache["nc"] = _build()
    nc = _cache["nc"]
    in_maps = _prep_inputs(x, *args)
    res = run_bass_kernel_spmd(nc, in_maps, list(range(8)))
    outf = np.empty((B, T, C), np.float32)
    for i in range(8):
        b, q0 = i // 4, (i % 4) * TQ
        outf[b, q0:q0 + TQ, :] = res.results[i]["out"]
    return outf


# revision 12
# speedup vs baseline: 1.2431x; 1.2431x over previous
"""Multi-head attention (B=2, T=4096, D=512, H=8) on 8 Trainium2 cores.

Sharding: core i handles batch b=i//4, query rows q0=(i%4)*1024 .. q0+1024,
all 8 heads (full K/V of its batch computed on-core; no collectives).
Host pre-transposes x and weights so every DMA is contiguous, and rolls
x along T per core so each core's query block sits at columns 0:1024
(keys become a permutation of T, which attention is invariant to).

Q/K/V projections run in bf16 (same PE throughput as fp32r, half the HBM
bytes); the O projection stays fp32r. The attention core (scores, attn@V)
runs in fp8e4m3 with DoubleRow perf mode (2 k-tiles per matmul at 0.5
cycles/row = 4x fp32r throughput):
  - q/k are scaled by alpha=8^-0.5 each (folds 1/sqrt(dk) into the
    quantization) and quantized to fp8; a partition-shuffle DMA packs
    them as [32, 2head, 2sub, T] so the 64-deep dk contraction becomes
    DoubleRow over 2x32.
  - exp fuses with fp8 quantization: one tensor_scalar per tile computes
    rne(score*8*log2e + 56) -> int8 whose bits ARE the fp8e4m3 encoding
    of exp(score) (Schraudolph in log2 space). Round-robins over
    ACT/Pool/DVE, splitting the 33.5M-exp/core load three ways.
  - attn@V contracts 256 keys per DoubleRow matmul against fp8 V packed
    [128, ktile, 96]: col 64 = ones so the softmax row-sum rides along
    as output partition 64; cols 65:95 pad the stationary width to a
    multiple of 32 (DoubleRow requirement) and are never read.
Scheduling: attention is software-pipelined (scores 2 k-tile-pairs ahead
of attn@V, 3x2-bank PSUM rotation) and independent background work (next
pair's projections / V / previous pair's normalize) is interleaved into
the attention stream so conversion latency plus the 100ns semaphore
delay never stall the in-order PE queue and the p-state ramp holds.
DMAs are coalesced via multi-dim access patterns to respect the serial
HWDGE dispatcher (~600ns each). Softmax skips max-subtraction (scores
~N(0,0.33)). Measured end-to-end rel err ~1.4e-2 vs the 2e-2 gate.
"""
import sys
sys.path.insert(0, "/opt/trn_rl_repo")

import numpy as np
import ml_dtypes
import concourse.bacc as bacc
import concourse.mybir as mybir
import concourse.tile as tile
from concourse.bass_utils import run_bass_kernel_spmd

F32 = mybir.dt.float32
F32R = mybir.dt.float32r
BF16 = mybir.dt.bfloat16
F8 = mybir.dt.float8e4
I8 = mybir.dt.int8
AF = mybir.ActivationFunctionType
MULT = mybir.AluOpType.mult
ADD = mybir.AluOpType.add
DR = mybir.MatmulPerfMode.DoubleRow

B, T, C = 2, 4096, 512
H, DK = 8, 64
TQ = 1024          # queries per core
NP = 4             # head pairs
KT = T // 128      # 32 k-tiles
CT = C // 128      # 4 contraction tiles
ALPHA = float(1.0 / np.sqrt(np.sqrt(64.0)))   # per-operand score scale
SCH_SCALE = 8 * 1.4426950408889634            # bits = rne(s*SCALE + 56)
SCH_BIAS = 56.0

_cache = {}


def _build():
    nc = bacc.Bacc("TRN2")
    xbT = nc.declare_dram_parameter("xbT", [C, T], BF16, isOutput=False)
    wqT = nc.declare_dram_parameter("wqT", [C, C], BF16, isOutput=False)
    wkT = nc.declare_dram_parameter("wkT", [C, C], BF16, isOutput=False)
    wvT = nc.declare_dram_parameter("wvT", [C, C], BF16, isOutput=False)
    woT = nc.declare_dram_parameter("woT", [C, C], F32R, isOutput=False)
    # bias[:, 0] = bq*ALPHA, bias[:, 1] = bk*ALPHA, bias[:, 2] = bv
    bias = nc.declare_dram_parameter("bias", [128, 3, NP], F32, isOutput=False)
    bo = nc.declare_dram_parameter("bo", [1, C], F32R, isOutput=False)
    # ind rows: 0 = head0 mask (1s in 0:64), 1 = head1 mask, 2 = all ones
    ind = nc.declare_dram_parameter("ind", [3, 128], F32R, isOutput=False)
    out = nc.declare_dram_parameter("out", [TQ, C], F32, isOutput=True)

    with tile.TileContext(nc) as tc:
        with (
            tc.tile_pool(name="big", bufs=1) as bpool,
            tc.tile_pool(name="const", bufs=1) as cpool,
            tc.tile_pool(name="work", bufs=2) as wpool,
            tc.tile_pool(name="k8p", bufs=2) as k8pool,
            tc.tile_pool(name="drp", bufs=2) as drpool,
            tc.tile_pool(name="attnp", bufs=3) as apool,
            tc.tile_pool(name="ps", bufs=3, space="PSUM") as ps,
            tc.tile_pool(name="psav", bufs=2, space="PSUM") as psav,
        ):
            def w_dma(tag, src, p0, width, dt=BF16):
                w = wpool.tile([128, CT, width], dt, tag=tag, name="w_" + tag)
                nc.sync.dma_start(
                    w[:], src[:, p0:p0 + width].rearrange(
                        "(ct p) c -> p ct c", p=128))
                return w

            # V weights + x first on the DMA queue: V-proj starts earliest
            wvs0 = w_dma("wvs", wvT, 0, 256)
            xT = bpool.tile([128, CT, T], BF16, tag="xT")          # 32KB
            nc.sync.dma_start(
                xT[:, :, 0:1024],
                xbT[:, 0:1024].rearrange("(ct p) t -> p ct t", p=128))
            bias_s = cpool.tile([128, 3, NP], F32, tag="bias")
            nc.sync.dma_start(bias_s[:], bias[:])
            for tch in range(1, 4):
                nc.sync.dma_start(
                    xT[:, :, tch * 1024:(tch + 1) * 1024],
                    xbT[:, tch * 1024:(tch + 1) * 1024].rearrange(
                        "(ct p) t -> p ct t", p=128))
            # ind / bo live at partition 64 so matmul operand bases match the
            # rowsum row (PSUM partition 64) they pair with.
            inds = cpool.tile([65, 3, 128], F32R, tag="ind")
            nc.sync.dma_start(inds[64:65, :, :],
                              ind.rearrange("(o a) b -> o a b", o=1))
            bos = cpool.tile([65, C], F32R, tag="bo")
            nc.sync.dma_start(bos[64:65, :], bo[:])
            sch_b = cpool.tile([128, 1], F32, tag="schb")
            nc.vector.memset(sch_b[:], SCH_BIAS)
            acat = bpool.tile([128, NP, TQ], F32, tag="acat")      # 16KB
            acat_r = acat.bitcast(F32R)

            # greedy least-loaded picker for vector-engine work.
            # cost(F) ~ F*cycle + fixed overhead, Pool at 0.6 efficiency
            load = {"act": 0.0, "pool": 0.0, "dve": 0.0}

            def pick_eng(free, exclude=()):
                costs = {"act": free * 0.833 + 180,
                         "dve": free * 1.042 + 110,
                         "pool": free * 1.389 + 135}
                e = min((x for x in costs if x not in exclude),
                        key=lambda x: load[x] + costs[x])
                load[e] += costs[e]
                return e

            def exp_quant(dst_i8, src_psum, free=1024):
                e = pick_eng(free)
                if e == "act":
                    nc.scalar.activation(dst_i8, src_psum, AF.Identity,
                                         bias=sch_b[:], scale=SCH_SCALE)
                elif e == "pool":
                    nc.gpsimd.tensor_scalar(dst_i8, src_psum, SCH_SCALE,
                                            SCH_BIAS, op0=MULT, op1=ADD)
                else:
                    nc.vector.tensor_scalar(dst_i8, src_psum, SCH_SCALE,
                                            SCH_BIAS, op0=MULT, op1=ADD)

            def g_copy(dst, srcv, free):
                e = pick_eng(free, exclude=("act",))
                eng = nc.vector if e == "dve" else nc.gpsimd
                eng.tensor_copy(dst, srcv)

            def qk_quant(dst, src_psum, bias_ap):
                e = pick_eng(1024)
                if e == "act":
                    nc.scalar.activation(dst, src_psum, AF.Identity,
                                         bias=bias_ap, scale=ALPHA)
                else:
                    eng = nc.vector if e == "dve" else nc.gpsimd
                    eng.tensor_scalar(dst, src_psum, bias_ap, ALPHA,
                                      op0=ADD, op1=MULT)

            # ---- V for one pair-group (4 heads): fp8, ones col at 64 ----
            def make_vproj(pg, wvs):
                v2p8 = bpool.tile([128, KT * 4, 96], F8, tag="v2p8",
                                  bufs=2, name="v2p8")             # 12KB x2
                nc.gpsimd.memset(v2p8[:, :, 64:96], 1.0)

                def gen():
                    for jp in range(KT // 2):
                        pv = ps.tile([128, 2, 512], F32, tag="pss",
                                     name="pv")
                        for jj in range(2):
                            j = jp * 2 + jj
                            for ct in range(CT):
                                nc.tensor.matmul(
                                    pv[:, jj, 0:256],
                                    xT[:, ct, j * 128:(j + 1) * 128],
                                    wvs[:, ct, :],
                                    start=(ct == 0), stop=(ct == CT - 1))
                        g_copy(
                            v2p8[:, jp * 8:(jp + 1) * 8, 0:64].rearrange(
                                "p (x a) b -> p x a b", x=2),
                            pv[:, :, 0:256].rearrange(
                                "p x (a b) -> p x a b", b=64), 512)
                        yield
                return v2p8, gen()

            def make_proj(p):
                wks = w_dma("wks", wkT, p * 128, 128)
                wqs = w_dma("wqs", wqT, p * 128, 128)
                k8 = k8pool.tile([128, T], F8, tag="k8", name="k8")
                q8 = k8pool.tile([128, TQ], F8, tag="q8", name="q8")
                # [32, h, i, t]: contraction index d = h*64 + i*32 + part
                kdr = drpool.tile([32, 2, 2, T], F8, tag="kdr", name="kdr")
                qdr = drpool.tile([32, 2, 2, TQ], F8, tag="qdr", name="qdr")

                def gen():
                    for t in range(4):
                        pk = ps.tile([128, 2, 512], F32, tag="pss", name="pk")
                        for half in range(2):
                            tch = t * 2 + half
                            for ct in range(CT):
                                nc.tensor.matmul(
                                    pk[:, half],
                                    wks[:, ct, :],
                                    xT[:, ct, tch * 512:(tch + 1) * 512],
                                    start=(ct == 0), stop=(ct == CT - 1))
                        qk_quant(k8[:, t * 1024:(t + 1) * 1024],
                                 pk[:].rearrange("p a b -> p (a b)"),
                                 bias_s[:, 1, p:p + 1])
                        yield
                    nc.sync.dma_start(
                        kdr[:], k8.rearrange("(g p) t -> p g t", p=32)
                        .rearrange("p (h i) t -> p h i t", h=2))
                    yield
                    pq = ps.tile([128, 2, 512], F32, tag="pss", name="pq")
                    for half in range(2):
                        for ct in range(CT):
                            nc.tensor.matmul(
                                pq[:, half],
                                wqs[:, ct, :],
                                xT[:, ct, half * 512:(half + 1) * 512],
                                start=(ct == 0), stop=(ct == CT - 1))
                    qk_quant(q8[:], pq[:].rearrange("p a b -> p (a b)"),
                             bias_s[:, 0, p:p + 1])
                    yield
                    nc.sync.dma_start(
                        qdr[:], q8.rearrange("(g p) t -> p g t", p=32)
                        .rearrange("p (h i) t -> p h i t", h=2))
                    yield
                return kdr, qdr, gen()

            def gen_attn(p, pi, kdr, qdr, v2p8, stage):
                DEPTH = 2
                avs = {}
                at8s = {}
                seq = [(hf, jp, h) for hf in range(2)
                       for jp in range(KT // 2) for h in range(2)]
                for idx in range(len(seq) + DEPTH):
                    if idx < len(seq):
                        half, jp, h = seq[idx]
                        at8 = apool.tile([128, 2, 512], I8, tag="attn",
                                         name="at8")
                        at8s[(half, jp, h)] = at8
                        pss = ps.tile([128, 2, 512], F32, tag="pss",
                                      name="pss")
                        for jj in range(2):
                            j = jp * 2 + jj
                            nc.tensor.matmul(
                                pss[:, jj],
                                kdr[:, h, :, j * 128:(j + 1) * 128],
                                qdr[:, h, :, half * 512:(half + 1) * 512],
                                start=True, stop=True, perf_mode=DR)
                        exp_quant(at8[:], pss[:])
                    if idx >= DEPTH:
                        half, jp, h = seq[idx - DEPTH]
                        c = pi * 2 + h
                        if jp == 0:
                            avs[(half, h)] = psav.tile([96, 512], F32,
                                                       tag="av", name="av")
                        at8 = at8s.pop((half, jp, h))
                        nc.tensor.matmul(
                            avs[(half, h)][:],
                            v2p8[:, jp * 8 + c:jp * 8 + c + 5:4, 0:96],
                            at8.bitcast(F8)[:],
                            start=(jp == 0), stop=(jp == KT // 2 - 1),
                            perf_mode=DR)
                        if jp == KT // 2 - 1:
                            av = avs.pop((half, h))
                            cols = slice(half * 512, (half + 1) * 512)
                            g_copy(stage[64:65, h, cols], av[64:65, :], 512)
                            if h == 0:
                                g_copy(acat[0:64, p, cols], av[0:64, :], 512)
                            else:
                                odd = wpool.tile([64, 512], F32,
                                                 tag="odd", name="odd")
                                g_copy(odd[:], av[0:64, :], 512)
                                nc.sync.dma_start(acat[64:128, p, cols],
                                                  odd[:])
                    yield

            def gen_norm(p, stage):
                # normalize: U / rowsum + bv  into acat[:, p, :]
                rb = wpool.tile([128, TQ], F32, tag="sc", name="rb")
                stage_r = stage.bitcast(F32R)
                pb = ps.tile([128, 2, 512], F32, tag="pss", name="pb")
                for half in range(2):
                    for h in range(2):
                        nc.tensor.matmul(
                            pb[:, half],
                            inds[64:65, h, :],
                            stage_r[64:65, h, half * 512:(half + 1) * 512],
                            start=(h == 0), stop=(h == 1))
                yield
                nc.vector.reciprocal(rb[:],
                                     pb[:].rearrange("p a b -> p (a b)"))
                yield
                e = pick_eng(1024, exclude=("act",))
                eng = nc.vector if e == "dve" else nc.gpsimd
                eng.tensor_tensor(
                    out=acat[:, p, :], in0=acat[:, p, :], in1=rb[:], op=MULT)
                yield
                e = pick_eng(1024, exclude=("act",))
                eng = nc.vector if e == "dve" else nc.gpsimd
                eng.tensor_scalar_add(
                    acat[:, p, :], acat[:, p, :], bias_s[:, 2, p:p + 1])
                yield

            def chain(*gens):
                for g in gens:
                    yield from g

            def run(gen, bg=None, ratio=4):
                i = 0
                for _ in gen:
                    i += 1
                    if bg is not None and i % ratio == 0:
                        next(bg, None)
                if bg is not None:
                    for _ in bg:
                        pass

            stages = [wpool.tile([65, 2, TQ], F32, tag="stage", bufs=4,
                                 name=f"stage{i}") for i in range(4)]

            # ---- schedule: V0 P0 P1 | A0+P2 | A1+(N0,V1) | A2+(N1,P3)
            #                | A3+N2 | N3 | O-proj ----
            v2p8_0, vg0 = make_vproj(0, wvs0)
            run(vg0)
            kdr0, qdr0, pg0 = make_proj(0)
            run(pg0)
            kdr1, qdr1, pg1 = make_proj(1)
            run(pg1)

            kdr2, qdr2, pg2 = make_proj(2)
            run(gen_attn(0, 0, kdr0, qdr0, v2p8_0, stages[0]), bg=pg2,
                ratio=9)
            wvs1 = w_dma("wvs", wvT, 256, 256)
            v2p8_1, vg1 = make_vproj(1, wvs1)
            run(gen_attn(1, 1, kdr1, qdr1, v2p8_0, stages[1]),
                bg=chain(gen_norm(0, stages[0]), vg1), ratio=3)
            kdr3, qdr3, pg3 = make_proj(3)
            run(gen_attn(2, 0, kdr2, qdr2, v2p8_1, stages[2]),
                bg=chain(gen_norm(1, stages[1]), pg3), ratio=5)
            woTs = cpool.tile([128, CT, C], F32R, tag="woT")       # 8KB
            nc.sync.dma_start(
                woTs[:], woT.rearrange("(ct p) c -> p ct c", p=128))
            run(gen_attn(3, 1, kdr3, qdr3, v2p8_1, stages[3]),
                bg=gen_norm(2, stages[2]), ratio=16)
            run(gen_norm(3, stages[3]))

            # ---- output projection: out[t, :] = acat^T.T @ woT + bo ----
            for t in range(4):
                po = ps.tile([128, 2, 512], F32, tag="pss", name="po")
                for half in range(2):
                    qt = t * 2 + half
                    for r in range(CT):
                        nc.tensor.matmul(
                            po[:, half],
                            acat_r[:, r, qt * 128:(qt + 1) * 128],
                            woTs[:, r, :],
                            start=(r == 0), stop=False)
                    nc.tensor.matmul(po[:, half], inds[64:65, 2, :],
                                     bos[64:65, :], start=False, stop=True)
                ot = wpool.tile([128, 2, C], F32, tag="sc", name="ot")
                g_copy(ot[:, 0, :], po[:, 0], 512)
                g_copy(ot[:, 1, :], po[:, 1], 512)
                nc.sync.dma_start(
                    out[t * 256:(t + 1) * 256, :].rearrange(
                        "(a p) c -> p a c", p=128), ot[:])

    nc.compile()
    return nc


def _prep_inputs(x, Wq, bq, Wk, bk, Wv, bv, Wo, bo):
    bf = ml_dtypes.bfloat16
    wqT = np.ascontiguousarray(Wq.T.astype(bf))
    wkT = np.ascontiguousarray(Wk.T.astype(bf))
    wvT = np.ascontiguousarray(Wv.T.astype(bf))
    woT = np.ascontiguousarray(Wo.T)
    bias = np.stack([
        (bq * ALPHA).reshape(NP, 128).T,
        (bk * ALPHA).reshape(NP, 128).T,
        bv.reshape(NP, 128).T,
    ], axis=1).astype(np.float32)          # [128, 3, NP]
    bias = np.ascontiguousarray(bias)
    bo1 = np.ascontiguousarray(bo.reshape(1, C))
    ind = np.zeros((3, 128), np.float32)
    ind[0, 0:64] = 1.0
    ind[1, 64:128] = 1.0
    ind[2, :] = 1.0
    in_maps = []
    for i in range(8):
        b, q0 = i // 4, (i % 4) * TQ
        xbT = np.ascontiguousarray(np.roll(x[b].T, -q0, axis=1).astype(bf))
        in_maps.append({
            "xbT": xbT, "wqT": wqT, "wkT": wkT, "wvT": wvT, "woT": woT,
            "bias": bias, "bo": bo1, "ind": ind,
        })
    return in_maps


def kernel(x, Wq, bq, Wk, bk, Wv, bv, Wo, bo):
    x = np.asarray(x, np.float32)
    args = [np.asarray(a, np.float32) for a in
            (Wq, bq, Wk, bk, Wv, bv, Wo, bo)]
    if "nc" not in _cache:
        _cache["nc"] = _build()
    nc = _cache["nc"]
    in_maps = _prep_inputs(x, *args)
    res = run_bass_kernel_spmd(nc, in_maps, list(range(8)))
    outf = np.empty((B, T, C), np.float32)
    for i in range(8):
        b, q0 = i // 4, (i % 4) * TQ
        outf[b, q0:q0 + TQ, :] = res.results[i]["out"]
    return outf


# revision 13
# speedup vs baseline: 1.2502x; 1.0057x over previous
"""Multi-head attention (B=2, T=4096, D=512, H=8) on 8 Trainium2 cores.

Sharding: core i handles batch b=i//4, query rows q0=(i%4)*1024 .. q0+1024,
all 8 heads (full K/V of its batch computed on-core; no collectives).
Host pre-transposes x and weights so every DMA is contiguous, and rolls
x along T per core so each core's query block sits at columns 0:1024
(keys become a permutation of T, which attention is invariant to).

Q/K/V projections run in bf16 (same PE throughput as fp32r, half the HBM
bytes); the O projection stays fp32r. The attention core (scores, attn@V)
runs in fp8e4m3 with DoubleRow perf mode (2 k-tiles per matmul at 0.5
cycles/row = 4x fp32r throughput):
  - q/k are scaled by alpha=8^-0.5 each (folds 1/sqrt(dk) into the
    quantization) and quantized to fp8; a partition-shuffle DMA packs
    them as [32, 2head, 2sub, T] so the 64-deep dk contraction becomes
    DoubleRow over 2x32.
  - exp fuses with fp8 quantization: one tensor_scalar per tile computes
    rne(score*8*log2e + 56) -> int8 whose bits ARE the fp8e4m3 encoding
    of exp(score) (Schraudolph in log2 space). Round-robins over
    ACT/Pool/DVE, splitting the 33.5M-exp/core load three ways.
  - attn@V contracts 256 keys per DoubleRow matmul against fp8 V packed
    [128, ktile, 96]: col 64 = ones so the softmax row-sum rides along
    as output partition 64; cols 65:95 pad the stationary width to a
    multiple of 32 (DoubleRow requirement) and are never read.
Scheduling: attention is software-pipelined (scores 2 k-tile-pairs ahead
of attn@V, 3x2-bank PSUM rotation) and independent background work (next
pair's projections / V / previous pair's normalize) is interleaved into
the attention stream so conversion latency plus the 100ns semaphore
delay never stall the in-order PE queue and the p-state ramp holds.
DMAs are coalesced via multi-dim access patterns to respect the serial
HWDGE dispatcher (~600ns each). Softmax skips max-subtraction (scores
~N(0,0.33)). Measured end-to-end rel err ~1.4e-2 vs the 2e-2 gate.
"""
import sys
sys.path.insert(0, "/opt/trn_rl_repo")

import numpy as np
import ml_dtypes
import concourse.bacc as bacc
import concourse.mybir as mybir
import concourse.tile as tile
from concourse.bass_utils import run_bass_kernel_spmd

F32 = mybir.dt.float32
F32R = mybir.dt.float32r
BF16 = mybir.dt.bfloat16
F8 = mybir.dt.float8e4
I8 = mybir.dt.int8
AF = mybir.ActivationFunctionType
MULT = mybir.AluOpType.mult
ADD = mybir.AluOpType.add
DR = mybir.MatmulPerfMode.DoubleRow

B, T, C = 2, 4096, 512
H, DK = 8, 64
TQ = 1024          # queries per core
NP = 4             # head pairs
KT = T // 128      # 32 k-tiles
CT = C // 128      # 4 contraction tiles
ALPHA = float(1.0 / np.sqrt(np.sqrt(64.0)))   # per-operand score scale
SCH_SCALE = 8 * 1.4426950408889634            # bits = rne(s*SCALE + 56)
SCH_BIAS = 56.0

_cache = {}


def _build():
    nc = bacc.Bacc("TRN2")
    xbT = nc.declare_dram_parameter("xbT", [C, T], BF16, isOutput=False)
    wqT = nc.declare_dram_parameter("wqT", [C, C], BF16, isOutput=False)
    wkT = nc.declare_dram_parameter("wkT", [C, C], BF16, isOutput=False)
    wvT = nc.declare_dram_parameter("wvT", [C, C], BF16, isOutput=False)
    woT = nc.declare_dram_parameter("woT", [C, C], F32R, isOutput=False)
    # bias[:, 0] = bq*ALPHA, bias[:, 1] = bk*ALPHA, bias[:, 2] = bv
    bias = nc.declare_dram_parameter("bias", [128, 3, NP], F32, isOutput=False)
    bo = nc.declare_dram_parameter("bo", [1, C], F32R, isOutput=False)
    # ind rows: 0 = head0 mask (1s in 0:64), 1 = head1 mask, 2 = all ones
    ind = nc.declare_dram_parameter("ind", [3, 128], F32R, isOutput=False)
    out = nc.declare_dram_parameter("out", [TQ, C], F32, isOutput=True)

    with tile.TileContext(nc) as tc:
        with (
            tc.tile_pool(name="big", bufs=1) as bpool,
            tc.tile_pool(name="const", bufs=1) as cpool,
            tc.tile_pool(name="work", bufs=2) as wpool,
            tc.tile_pool(name="k8p", bufs=2) as k8pool,
            tc.tile_pool(name="drp", bufs=2) as drpool,
            tc.tile_pool(name="attnp", bufs=4) as apool,
            tc.tile_pool(name="ps", bufs=3, space="PSUM") as ps,
            tc.tile_pool(name="psav", bufs=2, space="PSUM") as psav,
        ):
            def w_dma(tag, src, p0, width, dt=BF16):
                w = wpool.tile([128, CT, width], dt, tag=tag, name="w_" + tag)
                nc.sync.dma_start(
                    w[:], src[:, p0:p0 + width].rearrange(
                        "(ct p) c -> p ct c", p=128))
                return w

            # V weights + x first on the DMA queue: V-proj starts earliest
            wvs0 = w_dma("wvs", wvT, 0, 256)
            xT = bpool.tile([128, CT, T], BF16, tag="xT")          # 32KB
            nc.sync.dma_start(
                xT[:, :, 0:1024],
                xbT[:, 0:1024].rearrange("(ct p) t -> p ct t", p=128))
            bias_s = cpool.tile([128, 3, NP], F32, tag="bias")
            nc.sync.dma_start(bias_s[:], bias[:])
            for tch in range(1, 4):
                nc.sync.dma_start(
                    xT[:, :, tch * 1024:(tch + 1) * 1024],
                    xbT[:, tch * 1024:(tch + 1) * 1024].rearrange(
                        "(ct p) t -> p ct t", p=128))
            # ind / bo live at partition 64 so matmul operand bases match the
            # rowsum row (PSUM partition 64) they pair with.
            inds = cpool.tile([65, 3, 128], F32R, tag="ind")
            nc.sync.dma_start(inds[64:65, :, :],
                              ind.rearrange("(o a) b -> o a b", o=1))
            bos = cpool.tile([65, C], F32R, tag="bo")
            nc.sync.dma_start(bos[64:65, :], bo[:])
            sch_b = cpool.tile([128, 1], F32, tag="schb")
            nc.vector.memset(sch_b[:], SCH_BIAS)
            acat = bpool.tile([128, NP, TQ], F32, tag="acat")      # 16KB
            acat_r = acat.bitcast(F32R)

            # greedy least-loaded picker for vector-engine work.
            # cost(F) ~ F*cycle + fixed overhead, Pool at 0.6 efficiency
            load = {"act": 0.0, "pool": 0.0, "dve": 0.0}

            def pick_eng(free, exclude=()):
                costs = {"act": free * 0.833 + 180,
                         "dve": free * 1.042 + 110,
                         "pool": free * 1.389 + 135}
                e = min((x for x in costs if x not in exclude),
                        key=lambda x: load[x] + costs[x])
                load[e] += costs[e]
                return e

            def exp_quant(dst_i8, src_psum, free=1024):
                e = pick_eng(free)
                if e == "act":
                    nc.scalar.activation(dst_i8, src_psum, AF.Identity,
                                         bias=sch_b[:], scale=SCH_SCALE)
                elif e == "pool":
                    nc.gpsimd.tensor_scalar(dst_i8, src_psum, SCH_SCALE,
                                            SCH_BIAS, op0=MULT, op1=ADD)
                else:
                    nc.vector.tensor_scalar(dst_i8, src_psum, SCH_SCALE,
                                            SCH_BIAS, op0=MULT, op1=ADD)

            def g_copy(dst, srcv, free):
                e = pick_eng(free, exclude=("act",))
                eng = nc.vector if e == "dve" else nc.gpsimd
                eng.tensor_copy(dst, srcv)

            def qk_quant(dst, src_psum, bias_ap):
                e = pick_eng(1024)
                if e == "act":
                    nc.scalar.activation(dst, src_psum, AF.Identity,
                                         bias=bias_ap, scale=ALPHA)
                else:
                    eng = nc.vector if e == "dve" else nc.gpsimd
                    eng.tensor_scalar(dst, src_psum, bias_ap, ALPHA,
                                      op0=ADD, op1=MULT)

            # ---- V for one pair-group (4 heads): fp8, ones col at 64 ----
            def make_vproj(pg, wvs):
                v2p8 = bpool.tile([128, KT * 4, 96], F8, tag="v2p8",
                                  bufs=2, name="v2p8")             # 12KB x2
                nc.gpsimd.memset(v2p8[:, :, 64:96], 1.0)

                def gen():
                    for jp in range(KT // 2):
                        pv = ps.tile([128, 2, 512], F32, tag="pss",
                                     name="pv")
                        for jj in range(2):
                            j = jp * 2 + jj
                            for ct in range(CT):
                                nc.tensor.matmul(
                                    pv[:, jj, 0:256],
                                    xT[:, ct, j * 128:(j + 1) * 128],
                                    wvs[:, ct, :],
                                    start=(ct == 0), stop=(ct == CT - 1))
                        g_copy(
                            v2p8[:, jp * 8:(jp + 1) * 8, 0:64].rearrange(
                                "p (x a) b -> p x a b", x=2),
                            pv[:, :, 0:256].rearrange(
                                "p x (a b) -> p x a b", b=64), 512)
                        yield
                return v2p8, gen()

            def make_proj(p):
                wks = w_dma("wks", wkT, p * 128, 128)
                wqs = w_dma("wqs", wqT, p * 128, 128)
                k8 = k8pool.tile([128, T], F8, tag="k8", name="k8")
                q8 = k8pool.tile([128, TQ], F8, tag="q8", name="q8")
                # [32, h, i, t]: contraction index d = h*64 + i*32 + part
                kdr = drpool.tile([32, 2, 2, T], F8, tag="kdr", name="kdr")
                qdr = drpool.tile([32, 2, 2, TQ], F8, tag="qdr", name="qdr")

                def gen():
                    for t in range(4):
                        pk = ps.tile([128, 2, 512], F32, tag="pss", name="pk")
                        for half in range(2):
                            tch = t * 2 + half
                            for ct in range(CT):
                                nc.tensor.matmul(
                                    pk[:, half],
                                    wks[:, ct, :],
                                    xT[:, ct, tch * 512:(tch + 1) * 512],
                                    start=(ct == 0), stop=(ct == CT - 1))
                        qk_quant(k8[:, t * 1024:(t + 1) * 1024],
                                 pk[:].rearrange("p a b -> p (a b)"),
                                 bias_s[:, 1, p:p + 1])
                        yield
                    nc.sync.dma_start(
                        kdr[:], k8.rearrange("(g p) t -> p g t", p=32)
                        .rearrange("p (h i) t -> p h i t", h=2))
                    yield
                    pq = ps.tile([128, 2, 512], F32, tag="pss", name="pq")
                    for half in range(2):
                        for ct in range(CT):
                            nc.tensor.matmul(
                                pq[:, half],
                                wqs[:, ct, :],
                                xT[:, ct, half * 512:(half + 1) * 512],
                                start=(ct == 0), stop=(ct == CT - 1))
                    qk_quant(q8[:], pq[:].rearrange("p a b -> p (a b)"),
                             bias_s[:, 0, p:p + 1])
                    yield
                    nc.sync.dma_start(
                        qdr[:], q8.rearrange("(g p) t -> p g t", p=32)
                        .rearrange("p (h i) t -> p h i t", h=2))
                    yield
                return kdr, qdr, gen()

            def gen_attn(p, pi, kdr, qdr, v2p8, stage):
                DEPTH = 3
                avs = {}
                at8s = {}
                seq = [(hf, jp, h) for hf in range(2)
                       for jp in range(KT // 2) for h in range(2)]
                for idx in range(len(seq) + DEPTH):
                    if idx < len(seq):
                        half, jp, h = seq[idx]
                        at8 = apool.tile([128, 2, 512], I8, tag="attn",
                                         name="at8")
                        at8s[(half, jp, h)] = at8
                        pss = ps.tile([128, 2, 512], F32, tag="pss",
                                      name="pss")
                        for jj in range(2):
                            j = jp * 2 + jj
                            nc.tensor.matmul(
                                pss[:, jj],
                                kdr[:, h, :, j * 128:(j + 1) * 128],
                                qdr[:, h, :, half * 512:(half + 1) * 512],
                                start=True, stop=True, perf_mode=DR)
                        exp_quant(at8[:], pss[:])
                    if idx >= DEPTH:
                        half, jp, h = seq[idx - DEPTH]
                        c = pi * 2 + h
                        if jp == 0:
                            avs[(half, h)] = psav.tile([96, 512], F32,
                                                       tag="av", name="av")
                        at8 = at8s.pop((half, jp, h))
                        nc.tensor.matmul(
                            avs[(half, h)][:],
                            v2p8[:, jp * 8 + c:jp * 8 + c + 5:4, 0:96],
                            at8.bitcast(F8)[:],
                            start=(jp == 0), stop=(jp == KT // 2 - 1),
                            perf_mode=DR)
                        if jp == KT // 2 - 1:
                            av = avs.pop((half, h))
                            cols = slice(half * 512, (half + 1) * 512)
                            g_copy(stage[64:65, h, cols], av[64:65, :], 512)
                            if h == 0:
                                g_copy(acat[0:64, p, cols], av[0:64, :], 512)
                            else:
                                odd = wpool.tile([64, 512], F32,
                                                 tag="odd", name="odd")
                                g_copy(odd[:], av[0:64, :], 512)
                                nc.sync.dma_start(acat[64:128, p, cols],
                                                  odd[:])
                    yield

            def gen_norm(p, stage):
                # normalize: U / rowsum + bv  into acat[:, p, :]
                rb = wpool.tile([128, TQ], F32, tag="sc", name="rb")
                stage_r = stage.bitcast(F32R)
                pb = ps.tile([128, 2, 512], F32, tag="pss", name="pb")
                for half in range(2):
                    for h in range(2):
                        nc.tensor.matmul(
                            pb[:, half],
                            inds[64:65, h, :],
                            stage_r[64:65, h, half * 512:(half + 1) * 512],
                            start=(h == 0), stop=(h == 1))
                yield
                nc.vector.reciprocal(rb[:],
                                     pb[:].rearrange("p a b -> p (a b)"))
                yield
                e = pick_eng(1024, exclude=("act",))
                eng = nc.vector if e == "dve" else nc.gpsimd
                eng.tensor_tensor(
                    out=acat[:, p, :], in0=acat[:, p, :], in1=rb[:], op=MULT)
                yield
                e = pick_eng(1024, exclude=("act",))
                eng = nc.vector if e == "dve" else nc.gpsimd
                eng.tensor_scalar_add(
                    acat[:, p, :], acat[:, p, :], bias_s[:, 2, p:p + 1])
                yield

            def chain(*gens):
                for g in gens:
                    yield from g

            def run(gen, bg=None, ratio=4):
                i = 0
                for _ in gen:
                    i += 1
                    if bg is not None and i % ratio == 0:
                        next(bg, None)
                if bg is not None:
                    for _ in bg:
                        pass

            stages = [wpool.tile([65, 2, TQ], F32, tag="stage", bufs=4,
                                 name=f"stage{i}") for i in range(4)]

            # ---- schedule: V0 P0 P1 | A0+P2 | A1+(N0,V1) | A2+(N1,P3)
            #                | A3+N2 | N3 | O-proj ----
            v2p8_0, vg0 = make_vproj(0, wvs0)
            run(vg0)
            kdr0, qdr0, pg0 = make_proj(0)
            run(pg0)
            kdr1, qdr1, pg1 = make_proj(1)
            run(pg1)

            kdr2, qdr2, pg2 = make_proj(2)
            run(gen_attn(0, 0, kdr0, qdr0, v2p8_0, stages[0]), bg=pg2,
                ratio=9)
            wvs1 = w_dma("wvs", wvT, 256, 256)
            v2p8_1, vg1 = make_vproj(1, wvs1)
            run(gen_attn(1, 1, kdr1, qdr1, v2p8_0, stages[1]),
                bg=chain(gen_norm(0, stages[0]), vg1), ratio=3)
            kdr3, qdr3, pg3 = make_proj(3)
            run(gen_attn(2, 0, kdr2, qdr2, v2p8_1, stages[2]),
                bg=chain(gen_norm(1, stages[1]), pg3), ratio=5)
            woTs = cpool.tile([128, CT, C], F32R, tag="woT")       # 8KB
            nc.sync.dma_start(
                woTs[:], woT.rearrange("(ct p) c -> p ct c", p=128))
            run(gen_attn(3, 1, kdr3, qdr3, v2p8_1, stages[3]),
                bg=gen_norm(2, stages[2]), ratio=16)
            run(gen_norm(3, stages[3]))

            # ---- output projection: out[t, :] = acat^T.T @ woT + bo ----
            for t in range(4):
                po = ps.tile([128, 2, 512], F32, tag="pss", name="po")
                for half in range(2):
                    qt = t * 2 + half
                    for r in range(CT):
                        nc.tensor.matmul(
                            po[:, half],
                            acat_r[:, r, qt * 128:(qt + 1) * 128],
                            woTs[:, r, :],
                            start=(r == 0), stop=False)
                    nc.tensor.matmul(po[:, half], inds[64:65, 2, :],
                                     bos[64:65, :], start=False, stop=True)
                ot = wpool.tile([128, 2, C], F32, tag="sc", name="ot")
                g_copy(ot[:, 0, :], po[:, 0], 512)
                g_copy(ot[:, 1, :], po[:, 1], 512)
                nc.sync.dma_start(
                    out[t * 256:(t + 1) * 256, :].rearrange(
                        "(a p) c -> p a c", p=128), ot[:])

    nc.compile()
    return nc


def _prep_inputs(x, Wq, bq, Wk, bk, Wv, bv, Wo, bo):
    bf = ml_dtypes.bfloat16
    wqT = np.ascontiguousarray(Wq.T.astype(bf))
    wkT = np.ascontiguousarray(Wk.T.astype(bf))
    wvT = np.ascontiguousarray(Wv.T.astype(bf))
    woT = np.ascontiguousarray(Wo.T)
    bias = np.stack([
        (bq * ALPHA).reshape(NP, 128).T,
        (bk * ALPHA).reshape(NP, 128).T,
        bv.reshape(NP, 128).T,
    ], axis=1).astype(np.float32)          # [128, 3, NP]
    bias = np.ascontiguousarray(bias)
    bo1 = np.ascontiguousarray(bo.reshape(1, C))
    ind = np.zeros((3, 128), np.float32)
    ind[0, 0:64] = 1.0
    ind[1, 64:128] = 1.0
    ind[2, :] = 1.0
    in_maps = []
    for i in range(8):
        b, q0 = i // 4, (i % 4) * TQ
        xbT = np.ascontiguousarray(np.roll(x[b].T, -q0, axis=1).astype(bf))
        in_maps.append({
            "xbT": xbT, "wqT": wqT, "wkT": wkT, "wvT": wvT, "woT": woT,
            "bias": bias, "bo": bo1, "ind": ind,
        })
    return in_maps


def kernel(x, Wq, bq, Wk, bk, Wv, bv, Wo, bo):
    x = np.asarray(x, np.float32)
    args = [np.asarray(a, np.float32) for a in
            (Wq, bq, Wk, bk, Wv, bv, Wo, bo)]
    if "nc" not in _cache:
        _cache["nc"] = _build()
    nc = _cache["nc"]
    in_maps = _prep_inputs(x, *args)
    res = run_bass_kernel_spmd(nc, in_maps, list(range(8)))
    outf = np.empty((B, T, C), np.float32)
    for i in range(8):
        b, q0 = i // 4, (i % 4) * TQ
        outf[b, q0:q0 + TQ, :] = res.results[i]["out"]
    return outf


# revision 15
# speedup vs baseline: 1.3129x; 1.0501x over previous
"""Multi-head attention (B=2, T=4096, D=512, H=8) on 8 Trainium2 cores.

Sharding: core i handles batch b=i//4, query rows q0=(i%4)*1024 .. q0+1024,
all 8 heads (full K/V of its batch computed on-core; no collectives).
Host pre-transposes x and weights so every DMA is contiguous, and rolls
x along T per core so each core's query block sits at columns 0:1024
(keys become a permutation of T, which attention is invariant to).

Q/K/V projections run in bf16 (same PE throughput as fp32r, half the HBM
bytes); the O projection stays fp32r. The attention core (scores, attn@V)
runs in fp8e4m3 with DoubleRow perf mode (2 k-tiles per matmul at 0.5
cycles/row = 4x fp32r throughput):
  - q/k are scaled by alpha=8^-0.5 each (folds 1/sqrt(dk) into the
    quantization) and quantized to fp8; a partition-shuffle DMA packs
    them as [32, 2head, 2sub, T] so the 64-deep dk contraction becomes
    DoubleRow over 2x32.
  - exp fuses with fp8 quantization: one tensor_scalar per tile computes
    rne(score*8*log2e + 56) -> int8 whose bits ARE the fp8e4m3 encoding
    of exp(score) (Schraudolph in log2 space). Round-robins over
    ACT/Pool/DVE, splitting the 33.5M-exp/core load three ways.
  - attn@V contracts 256 keys per DoubleRow matmul against fp8 V packed
    [128, ktile, 96]: col 64 = ones so the softmax row-sum rides along
    as output partition 64; cols 65:95 pad the stationary width to a
    multiple of 32 (DoubleRow requirement) and are never read.
Scheduling: attention is software-pipelined (scores 2 k-tile-pairs ahead
of attn@V, 3x2-bank PSUM rotation) and independent background work (next
pair's projections / V / previous pair's normalize) is interleaved into
the attention stream so conversion latency plus the 100ns semaphore
delay never stall the in-order PE queue and the p-state ramp holds.
DMAs are coalesced via multi-dim access patterns to respect the serial
HWDGE dispatcher (~600ns each). Softmax skips max-subtraction (scores
~N(0,0.33)). Measured end-to-end rel err ~1.4e-2 vs the 2e-2 gate.
"""
import sys
sys.path.insert(0, "/opt/trn_rl_repo")

import numpy as np
import ml_dtypes
import concourse.bacc as bacc
import concourse.mybir as mybir
import concourse.tile as tile
from concourse.bass_utils import run_bass_kernel_spmd

F32 = mybir.dt.float32
F32R = mybir.dt.float32r
BF16 = mybir.dt.bfloat16
F8 = mybir.dt.float8e4
I8 = mybir.dt.int8
AF = mybir.ActivationFunctionType
MULT = mybir.AluOpType.mult
ADD = mybir.AluOpType.add
DR = mybir.MatmulPerfMode.DoubleRow

B, T, C = 2, 4096, 512
H, DK = 8, 64
TQ = 1024          # queries per core
NP = 4             # head pairs
KT = T // 128      # 32 k-tiles
CT = C // 128      # 4 contraction tiles
ALPHA = float(1.0 / np.sqrt(np.sqrt(64.0)))   # per-operand score scale
SCH_SCALE = 8 * 1.4426950408889634            # bits = rne(s*SCALE + 56)
SCH_BIAS = 56.0

_cache = {}


def _build():
    nc = bacc.Bacc("TRN2")
    xbT = nc.declare_dram_parameter("xbT", [C, T], BF16, isOutput=False)
    wqT = nc.declare_dram_parameter("wqT", [C, C], BF16, isOutput=False)
    wkT = nc.declare_dram_parameter("wkT", [C, C], BF16, isOutput=False)
    wvT = nc.declare_dram_parameter("wvT", [C, C], BF16, isOutput=False)
    woT = nc.declare_dram_parameter("woT", [C, C], F32R, isOutput=False)
    # bias[:, 0] = bq*ALPHA, bias[:, 1] = bk*ALPHA, bias[:, 2] = bv
    bias = nc.declare_dram_parameter("bias", [128, 3, NP], F32, isOutput=False)
    bo = nc.declare_dram_parameter("bo", [1, C], F32R, isOutput=False)
    # ind rows: 0 = head0 mask (1s in 0:64), 1 = head1 mask, 2 = all ones
    ind = nc.declare_dram_parameter("ind", [3, 128], F32R, isOutput=False)
    out = nc.declare_dram_parameter("out", [TQ, C], F32, isOutput=True)

    with tile.TileContext(nc) as tc:
        with (
            tc.tile_pool(name="big", bufs=1) as bpool,
            tc.tile_pool(name="const", bufs=1) as cpool,
            tc.tile_pool(name="work", bufs=2) as wpool,
            tc.tile_pool(name="k8p", bufs=2) as k8pool,
            tc.tile_pool(name="drp", bufs=2) as drpool,
            tc.tile_pool(name="attnp", bufs=4) as apool,
            tc.tile_pool(name="ps", bufs=3, space="PSUM") as ps,
            tc.tile_pool(name="psav", bufs=2, space="PSUM") as psav,
        ):
            def w_dma(tag, src, p0, width, dt=BF16):
                w = wpool.tile([128, CT, width], dt, tag=tag, name="w_" + tag)
                nc.sync.dma_start(
                    w[:], src[:, p0:p0 + width].rearrange(
                        "(ct p) c -> p ct c", p=128))
                return w

            # V weights + x first on the DMA queue: V-proj starts earliest
            wvs0 = w_dma("wvs", wvT, 0, 256)
            xT = bpool.tile([128, CT, T], BF16, tag="xT")          # 32KB
            nc.sync.dma_start(
                xT[:, :, 0:1024],
                xbT[:, 0:1024].rearrange("(ct p) t -> p ct t", p=128))
            bias_s = cpool.tile([128, 3, NP], F32, tag="bias")
            nc.sync.dma_start(bias_s[:], bias[:])
            for tch in range(1, 4):
                nc.sync.dma_start(
                    xT[:, :, tch * 1024:(tch + 1) * 1024],
                    xbT[:, tch * 1024:(tch + 1) * 1024].rearrange(
                        "(ct p) t -> p ct t", p=128))
            # ind / bo live at partition 64 so matmul operand bases match the
            # rowsum row (PSUM partition 64) they pair with.
            inds = cpool.tile([65, 3, 128], F32R, tag="ind")
            nc.sync.dma_start(inds[64:65, :, :],
                              ind.rearrange("(o a) b -> o a b", o=1))
            bos = cpool.tile([65, C], F32R, tag="bo")
            nc.sync.dma_start(bos[64:65, :], bo[:])
            sch_b = cpool.tile([128, 1], F32, tag="schb")
            nc.vector.memset(sch_b[:], SCH_BIAS)
            acat = bpool.tile([128, NP, TQ], F32, tag="acat")      # 16KB
            acat_r = acat.bitcast(F32R)

            # greedy least-loaded picker. Pool (GPSIMD) cannot touch PSUM
            # on TRN2, so all PSUM-sourced ops go ACT/DVE only.
            load = {"act": 0.0, "dve": 0.0}

            def pick_eng(free, exclude=()):
                costs = {"act": free * 0.833 + 180,
                         "dve": free * 1.042 + 110}
                e = min((x for x in costs if x not in exclude),
                        key=lambda x: load[x] + costs[x])
                load[e] += costs[e]
                return e

            def exp_quant(dst_i8, src_psum, free=1024):
                e = pick_eng(free)
                if e == "act":
                    nc.scalar.activation(dst_i8, src_psum, AF.Identity,
                                         bias=sch_b[:], scale=SCH_SCALE)
                else:
                    nc.vector.tensor_scalar(dst_i8, src_psum, SCH_SCALE,
                                            SCH_BIAS, op0=MULT, op1=ADD)

            def g_copy(dst, srcv, free):
                e = pick_eng(free)
                if e == "act":
                    nc.scalar.copy(dst, srcv)
                else:
                    nc.vector.tensor_copy(dst, srcv)

            def qk_quant(dst, src_psum, bias_ap):
                e = pick_eng(1024)
                if e == "act":
                    nc.scalar.activation(dst, src_psum, AF.Identity,
                                         bias=bias_ap, scale=ALPHA)
                else:
                    nc.vector.tensor_scalar(dst, src_psum, bias_ap, ALPHA,
                                            op0=ADD, op1=MULT)

            # ---- V for one pair-group (4 heads): fp8, ones col at 64 ----
            def make_vproj(pg, wvs):
                v2p8 = bpool.tile([128, KT * 4, 96], F8, tag="v2p8",
                                  bufs=2, name="v2p8")             # 12KB x2
                nc.gpsimd.memset(v2p8[:, :, 64:96], 1.0)

                def gen():
                    for jp in range(KT // 2):
                        pv = ps.tile([128, 2, 512], F32, tag="pss",
                                     name="pv")
                        for jj in range(2):
                            j = jp * 2 + jj
                            for ct in range(CT):
                                nc.tensor.matmul(
                                    pv[:, jj, 0:256],
                                    xT[:, ct, j * 128:(j + 1) * 128],
                                    wvs[:, ct, :],
                                    start=(ct == 0), stop=(ct == CT - 1))
                        g_copy(
                            v2p8[:, jp * 8:(jp + 1) * 8, 0:64].rearrange(
                                "p (x a) b -> p x a b", x=2),
                            pv[:, :, 0:256].rearrange(
                                "p x (a b) -> p x a b", b=64), 512)
                        yield
                return v2p8, gen()

            def make_proj(p):
                wks = w_dma("wks", wkT, p * 128, 128)
                wqs = w_dma("wqs", wqT, p * 128, 128)
                k8 = k8pool.tile([128, T], F8, tag="k8", name="k8")
                q8 = k8pool.tile([128, TQ], F8, tag="q8", name="q8")
                # [32, h, i, t]: contraction index d = h*64 + i*32 + part
                kdr = drpool.tile([32, 2, 2, T], F8, tag="kdr", name="kdr")
                qdr = drpool.tile([32, 2, 2, TQ], F8, tag="qdr", name="qdr")

                def gen():
                    for t in range(4):
                        pk = ps.tile([128, 2, 512], F32, tag="pss", name="pk")
                        for half in range(2):
                            tch = t * 2 + half
                            for ct in range(CT):
                                nc.tensor.matmul(
                                    pk[:, half],
                                    wks[:, ct, :],
                                    xT[:, ct, tch * 512:(tch + 1) * 512],
                                    start=(ct == 0), stop=(ct == CT - 1))
                        qk_quant(k8[:, t * 1024:(t + 1) * 1024],
                                 pk[:].rearrange("p a b -> p (a b)"),
                                 bias_s[:, 1, p:p + 1])
                        yield
                    for g in range(4):
                        nc.sync.dma_start(kdr[:, g // 2, g % 2, :],
                                          k8[g * 32:(g + 1) * 32, :])
                    yield
                    pq = ps.tile([128, 2, 512], F32, tag="pss", name="pq")
                    for half in range(2):
                        for ct in range(CT):
                            nc.tensor.matmul(
                                pq[:, half],
                                wqs[:, ct, :],
                                xT[:, ct, half * 512:(half + 1) * 512],
                                start=(ct == 0), stop=(ct == CT - 1))
                    qk_quant(q8[:], pq[:].rearrange("p a b -> p (a b)"),
                             bias_s[:, 0, p:p + 1])
                    yield
                    for g in range(4):
                        nc.sync.dma_start(qdr[:, g // 2, g % 2, :],
                                          q8[g * 32:(g + 1) * 32, :])
                    yield
                return kdr, qdr, gen()

            def gen_attn(p, pi, kdr, qdr, v2p8, stage):
                DEPTH = 3
                avs = {}
                at8s = {}
                seq = [(hf, jp, h) for hf in range(2)
                       for jp in range(KT // 2) for h in range(2)]
                for idx in range(len(seq) + DEPTH):
                    if idx < len(seq):
                        half, jp, h = seq[idx]
                        at8 = apool.tile([128, 2, 512], I8, tag="attn",
                                         name="at8")
                        at8s[(half, jp, h)] = at8
                        pss = ps.tile([128, 2, 512], F32, tag="pss",
                                      name="pss")
                        for jj in range(2):
                            j = jp * 2 + jj
                            nc.tensor.matmul(
                                pss[:, jj],
                                kdr[:, h, :, j * 128:(j + 1) * 128],
                                qdr[:, h, :, half * 512:(half + 1) * 512],
                                start=True, stop=True, perf_mode=DR)
                        exp_quant(at8[:], pss[:])
                    if idx >= DEPTH:
                        half, jp, h = seq[idx - DEPTH]
                        c = pi * 2 + h
                        if jp == 0:
                            avs[(half, h)] = psav.tile([96, 512], F32,
                                                       tag="av", name="av")
                        at8 = at8s.pop((half, jp, h))
                        nc.tensor.matmul(
                            avs[(half, h)][:],
                            v2p8[:, jp * 8 + c:jp * 8 + c + 5:4, 0:96],
                            at8.bitcast(F8)[:],
                            start=(jp == 0), stop=(jp == KT // 2 - 1),
                            perf_mode=DR)
                        if jp == KT // 2 - 1:
                            av = avs.pop((half, h))
                            cols = slice(half * 512, (half + 1) * 512)
                            dr = wpool.tile([65, 512], F32, tag="odd",
                                            bufs=4, name="dr")
                            g_copy(dr[:], av[0:65, :], 512)
                            nc.sync.dma_start(stage[64:65, h, cols],
                                              dr[64:65, :])
                            dst = (acat[0:64, p, cols] if h == 0
                                   else acat[64:128, p, cols])
                            nc.sync.dma_start(dst, dr[0:64, :])
                    yield

            def gen_norm(p, stage):
                # normalize: U / rowsum + bv  into acat[:, p, :]
                rb = wpool.tile([128, TQ], F32, tag="sc", name="rb")
                stage_r = stage.bitcast(F32R)
                pb = ps.tile([128, 2, 512], F32, tag="pss", name="pb")
                for half in range(2):
                    for h in range(2):
                        nc.tensor.matmul(
                            pb[:, half],
                            inds[64:65, h, :],
                            stage_r[64:65, h, half * 512:(half + 1) * 512],
                            start=(h == 0), stop=(h == 1))
                yield
                nc.vector.reciprocal(rb[:],
                                     pb[:].rearrange("p a b -> p (a b)"))
                yield
                nc.gpsimd.tensor_tensor(
                    out=acat[:, p, :], in0=acat[:, p, :], in1=rb[:], op=MULT)
                yield
                nc.gpsimd.tensor_scalar_add(
                    acat[:, p, :], acat[:, p, :], bias_s[:, 2, p:p + 1])
                yield

            def chain(*gens):
                for g in gens:
                    yield from g

            def run(gen, bg=None, ratio=4):
                i = 0
                for _ in gen:
                    i += 1
                    if bg is not None and i % ratio == 0:
                        next(bg, None)
                if bg is not None:
                    for _ in bg:
                        pass

            stages = [wpool.tile([65, 2, TQ], F32, tag="stage", bufs=4,
                                 name=f"stage{i}") for i in range(4)]

            # ---- schedule: V0 P0 P1 | A0+P2 | A1+(N0,V1) | A2+(N1,P3)
            #                | A3+N2 | N3 | O-proj ----
            v2p8_0, vg0 = make_vproj(0, wvs0)
            run(vg0)
            kdr0, qdr0, pg0 = make_proj(0)
            run(pg0)
            kdr1, qdr1, pg1 = make_proj(1)
            run(pg1)

            kdr2, qdr2, pg2 = make_proj(2)
            run(gen_attn(0, 0, kdr0, qdr0, v2p8_0, stages[0]), bg=pg2,
                ratio=9)
            wvs1 = w_dma("wvs", wvT, 256, 256)
            v2p8_1, vg1 = make_vproj(1, wvs1)
            run(gen_attn(1, 1, kdr1, qdr1, v2p8_0, stages[1]),
                bg=chain(gen_norm(0, stages[0]), vg1), ratio=3)
            kdr3, qdr3, pg3 = make_proj(3)
            run(gen_attn(2, 0, kdr2, qdr2, v2p8_1, stages[2]),
                bg=chain(gen_norm(1, stages[1]), pg3), ratio=5)
            woTs = cpool.tile([128, CT, C], F32R, tag="woT")       # 8KB
            nc.sync.dma_start(
                woTs[:], woT.rearrange("(ct p) c -> p ct c", p=128))
            run(gen_attn(3, 1, kdr3, qdr3, v2p8_1, stages[3]),
                bg=gen_norm(2, stages[2]), ratio=16)
            run(gen_norm(3, stages[3]))

            # ---- output projection: out[t, :] = acat^T.T @ woT + bo ----
            for t in range(4):
                po = ps.tile([128, 2, 512], F32, tag="pss", name="po")
                for half in range(2):
                    qt = t * 2 + half
                    for r in range(CT):
                        nc.tensor.matmul(
                            po[:, half],
                            acat_r[:, r, qt * 128:(qt + 1) * 128],
                            woTs[:, r, :],
                            start=(r == 0), stop=False)
                    nc.tensor.matmul(po[:, half], inds[64:65, 2, :],
                                     bos[64:65, :], start=False, stop=True)
                ot = wpool.tile([128, 2, C], F32, tag="sc", name="ot")
                g_copy(ot[:, 0, :], po[:, 0], 512)
                g_copy(ot[:, 1, :], po[:, 1], 512)
                nc.sync.dma_start(
                    out[t * 256:(t + 1) * 256, :].rearrange(
                        "(a p) c -> p a c", p=128), ot[:])

    nc.compile()
    return nc


def _prep_inputs(x, Wq, bq, Wk, bk, Wv, bv, Wo, bo):
    bf = ml_dtypes.bfloat16
    wqT = np.ascontiguousarray(Wq.T.astype(bf))
    wkT = np.ascontiguousarray(Wk.T.astype(bf))
    wvT = np.ascontiguousarray(Wv.T.astype(bf))
    woT = np.ascontiguousarray(Wo.T)
    bias = np.stack([
        (bq * ALPHA).reshape(NP, 128).T,
        (bk * ALPHA).reshape(NP, 128).T,
        bv.reshape(NP, 128).T,
    ], axis=1).astype(np.float32)          # [128, 3, NP]
    bias = np.ascontiguousarray(bias)
    bo1 = np.ascontiguousarray(bo.reshape(1, C))
    ind = np.zeros((3, 128), np.float32)
    ind[0, 0:64] = 1.0
    ind[1, 64:128] = 1.0
    ind[2, :] = 1.0
    in_maps = []
    for i in range(8):
        b, q0 = i // 4, (i % 4) * TQ
        xbT = np.ascontiguousarray(np.roll(x[b].T, -q0, axis=1).astype(bf))
        in_maps.append({
            "xbT": xbT, "wqT": wqT, "wkT": wkT, "wvT": wvT, "woT": woT,
            "bias": bias, "bo": bo1, "ind": ind,
        })
    return in_maps


def kernel(x, Wq, bq, Wk, bk, Wv, bv, Wo, bo):
    x = np.asarray(x, np.float32)
    args = [np.asarray(a, np.float32) for a in
            (Wq, bq, Wk, bk, Wv, bv, Wo, bo)]
    if "nc" not in _cache:
        _cache["nc"] = _build()
    nc = _cache["nc"]
    in_maps = _prep_inputs(x, *args)
    res = run_bass_kernel_spmd(nc, in_maps, list(range(8)))
    outf = np.empty((B, T, C), np.float32)
    for i in range(8):
        b, q0 = i // 4, (i % 4) * TQ
        outf[b, q0:q0 + TQ, :] = res.results[i]["out"]
    return outf
